# revision 10
# baseline (speedup 1.0000x reference)
"""Trainium2 Bass kernel for nn_ETCContext (sparse_attention).

Math (per batch b):
  k,q,v = split heads of K,Q,V               [H, L, d]
  cw[h,l,m] = sum_d k[h,l,d] q[h,m,d]        [H, L, L]
  LayerNorm over h:  (cw - mu)/sqrt(var+eps) * gamma[h] + beta[h]
  masked softmax over m (pad_mask = valid[l] & valid[m]), NaN->0
  out[l, h*64+d] = sum_m cw[h,l,m] v[h,m,d]
  w[m] = softmax_m( mask(sum_{h,l} cw[h,l,m] / (H*doc)) )

Sharding: 8 cores = (batch b = core//2) x (l-half = core%2).  Each core
computes its l-slice of the score tensor [H, 512, 1024] fully on-chip:
  PE:  QK^T per head, mean-over-heads as a stacked K=512 matmul (kT/8),
       PE transposes e' -> pT, AV matmul (with a ones-column in V giving
       the softmax denominator as output row 64), and the w-reduction
       as matmuls with r=1/s as the stationary 1-column weight.
  DVE: centering (c - mu) fused with PSUM evac (scalar_tensor_tensor),
       squares/sum for variance, z = chat*rs, mask multiply fused with
       the per-row softmax denominator accumulation (accum_out).
  ACT: mu evac, half the squares, rsqrt via Ln+Exp (same table set as
       the softmax Exp -> no table thrash), exp(gamma*z+beta), evacs.
Host: slice/transpose inputs per core, final cheap unshard (divide by
the denominators, zero padded rows, and the [B,1024] w softmax).
"""

import numpy as np
import ml_dtypes

B, L, D, H = 4, 1024, 512, 8
DH = D // H  # 64
NCORES = 8
LCORE = L // 2   # 512 l-rows per core
NLC = LCORE // 128  # 4 l-chunks
BF = ml_dtypes.bfloat16

_NC = None
_PATCHED = False


def _install_multiwait_split():
    """walrus in this env rejects >1 on_wait per instruction; split extras
    onto NoOp wait-carriers on the same engine (what raw bass would emit)."""
    global _PATCHED
    if _PATCHED:
        return
    _PATCHED = True
    import json
    import concourse.bass as bass

    orig = bass.Bass.to_json_bytes

    def to_json_bytes_split(self, *a, **kw):
        bir = orig(self, *a, **kw)
        j = json.loads(bir)
        ctr = 0
        changed = False
        for f in j.get("functions", []):
            for bb in f.get("blocks", []):
                out = []
                for inst in bb["instructions"]:
                    si = inst.get("sync_info")
                    if si:
                        w = si.get("on_wait") or []
                        if len(w) > 1:
                            changed = True
                            for extra in w[:-1]:
                                ctr += 1
                                out.append({
                                    "debug": inst.get("debug", 0),
                                    "engine": inst["engine"],
                                    "ins": [], "outs": [],
                                    "name": f"I-wsplit-{ctr}",
                                    "opcode": "NoOp",
                                    "text_hint": "wsplit",
                                    "sync_info": {"on_wait": [extra],
                                                  "on_update": []},
                                })
                            si["on_wait"] = [w[-1]]
                    out.append(inst)
                bb["instructions"] = out
        if not changed:
            return bir
        return json.dumps(j).encode()

    bass.Bass.to_json_bytes = to_json_bytes_split


def _build_nc():
    _install_multiwait_split()
    import concourse.bass as bass
    import concourse.mybir as mybir
    import concourse.tile as tile

    f32 = mybir.dt.float32
    bf16 = mybir.dt.bfloat16
    ALU = mybir.AluOpType
    ACT = mybir.ActivationFunctionType

    nc = bass.Bass(trn_type="TRN2")

    # qT | kT | kT8 packed along the free dim: [D, L + 2*LCORE]
    c32 = nc.declare_dram_parameter("c32", [D, L + 2 * LCORE], f32, isOutput=False)
    vx = nc.declare_dram_parameter("vx", [L, H * 65], bf16, isOutput=False)
    maskb = nc.declare_dram_parameter("maskb", [128, L], bf16, isOutput=False)
    vl = nc.declare_dram_parameter("vl", [128, NLC], f32, isOutput=False)
    gb = nc.declare_dram_parameter("gb", [128, 2 * H + 1], f32, isOutput=False)
    identb = nc.declare_dram_parameter("identb", [128, 128], bf16, isOutput=False)
    o_d = nc.declare_dram_parameter("o", [H, 65, LCORE], f32, isOutput=True)
    wnum_d = nc.declare_dram_parameter("wnum", [1, L], f32, isOutput=True)

    with tile.TileContext(nc) as tc:
        with (
            tc.tile_pool(name="const", bufs=1) as constp,
            tc.tile_pool(name="epool", bufs=1) as epool,
            tc.tile_pool(name="sqp", bufs=1) as sqp,
            tc.tile_pool(name="work", bufs=2) as work,
            tc.tile_pool(name="psA", bufs=2, space=bass.MemorySpace.PSUM) as psA,
            tc.tile_pool(name="psPT", bufs=1, space=bass.MemorySpace.PSUM) as psPT,
            tc.tile_pool(name="psAV", bufs=1, space=bass.MemorySpace.PSUM) as psAV,
            tc.tile_pool(name="psW", bufs=1, space=bass.MemorySpace.PSUM) as psW,
        ):
            # ---- static loads ----
            RB = L + 2 * LCORE  # 2048: row-block stride of packed f32 consts
            c32s = constp.tile([128, 4 * RB], f32, tag="c32s")
            nc.gpsimd.dma_start(c32s[:].rearrange("p (kc x) -> p kc x", kc=4),
                                c32.rearrange("(kc p) x -> p kc x", p=128))
            vxs = constp.tile([128, 8 * H * 65], bf16, tag="vxs")
            nc.gpsimd.dma_start(vxs[:].rearrange("p (mc j) -> p mc j", mc=8),
                              vx.rearrange("(mc p) j -> p mc j", p=128))
            maskbt = constp.tile([128, L], bf16, tag="maskbt")
            nc.gpsimd.dma_start(maskbt[:], maskb[:])
            vlt = constp.tile([128, NLC], f32, tag="vlt")
            nc.gpsimd.dma_start(vlt[:], vl[:])
            gbt = constp.tile([128, 2 * H + 1], f32, tag="gbt")
            nc.gpsimd.dma_start(gbt[:], gb[:])
            idt = constp.tile([128, 128], bf16, tag="idt")
            nc.gpsimd.dma_start(idt[:], identb[:])

            # persistent per-head masked-exp tiles: ep[h][p, lc*L + m] = e'(l=lc*128+p, m)
            ep = [epool.tile([128, NLC * L], bf16, tag=f"e{h}", name=f"ep{h}")
                  for h in range(H)]
            scol = constp.tile([128, H * NLC], f32, tag="scol")
            o_sb = [work.tile([65, LCORE], f32, tag=f"osb{h % 4}", name=f"osb{h}")
                    for h in range(H)]

            def qk_lhsT(h, lc):
                base = (h // 2) * RB + L + lc * 128
                return c32s[(h % 2) * 64:(h % 2) * 64 + 64, base:base + 128]

            def qk_rhs(h, mb):
                base = (h // 2) * RB + mb * 512
                return c32s[(h % 2) * 64:(h % 2) * 64 + 64, base:base + 512]

            for lc in range(NLC):
                # mean over heads via stacked matmul with kT/8 (K = 512, 4 chunks)
                mu_ps = psA.tile([128, L], f32, tag="cmu")
                for mb in range(2):
                    for kc in range(4):
                        nc.tensor.matmul(
                            mu_ps[:, mb * 512:(mb + 1) * 512],
                            c32s[:, kc * RB + L + LCORE + lc * 128: kc * RB + L + LCORE + lc * 128 + 128],
                            c32s[:, kc * RB + mb * 512: kc * RB + mb * 512 + 512],
                            start=(kc == 0), stop=(kc == 3),
                        )
                mu_s = work.tile([128, L], f32, tag="mus")
                nc.scalar.copy(mu_s[:], mu_ps[:])

                sqs = []
                for h in range(H):
                    c_ps = psA.tile([128, L], f32, tag="cmu")
                    for mb in range(2):
                        nc.tensor.matmul(
                            c_ps[:, mb * 512:(mb + 1) * 512],
                            qk_lhsT(h, lc), qk_rhs(h, mb),
                            start=True, stop=True,
                        )
                    # chat = c - mu  (evac fused), staged into the ep slot
                    ch = ep[h][:, lc * L:(lc + 1) * L]
                    nc.vector.scalar_tensor_tensor(
                        ch, c_ps[:], 0.0, mu_s[:], ALU.bypass, ALU.subtract)
                    sq = sqp.tile([128, L], bf16, tag=f"sq{h}")
                    if h % 2 == 0:
                        nc.scalar.activation(sq[:], ch, ACT.Square)
                    else:
                        nc.vector.tensor_tensor(sq[:], ch, ch, ALU.mult)
                    sqs.append(sq)

                # S2 = sum_h sq_h  (pairwise tree)
                t0 = work.tile([128, L], bf16, tag="ta")
                t1 = work.tile([128, L], bf16, tag="tb")
                t2 = work.tile([128, L], bf16, tag="tc")
                t3 = work.tile([128, L], bf16, tag="ta")
                nc.vector.tensor_tensor(t0[:], sqs[0][:], sqs[1][:], ALU.add)
                nc.vector.tensor_tensor(t1[:], sqs[2][:], sqs[3][:], ALU.add)
                nc.vector.tensor_tensor(t2[:], sqs[4][:], sqs[5][:], ALU.add)
                nc.vector.tensor_tensor(t3[:], sqs[6][:], sqs[7][:], ALU.add)
                u0 = work.tile([128, L], bf16, tag="tb")
                u1 = work.tile([128, L], bf16, tag="tc")
                nc.vector.tensor_tensor(u0[:], t0[:], t1[:], ALU.add)
                nc.vector.tensor_tensor(u1[:], t2[:], t3[:], ALU.add)
                s2 = work.tile([128, L], bf16, tag="s2")
                nc.vector.tensor_tensor(s2[:], u0[:], u1[:], ALU.add)

                # rs = (S2/8 + eps)^(-1/2) via Ln+Exp (same ACT table set as Exp)
                lnt = work.tile([128, L], f32, tag="lnt")
                nc.scalar.activation(lnt[:], s2[:], ACT.Ln, bias=gbt[:, 2 * H:2 * H + 1], scale=0.125)
                rs = work.tile([128, L], bf16, tag="rs")
                nc.scalar.activation(rs[:], lnt[:], ACT.Exp, bias=0.0, scale=-0.5)

                for h in range(H):
                    ch = ep[h][:, lc * L:(lc + 1) * L]
                    z = work.tile([128, L], bf16, tag="z")
                    nc.vector.scalar_tensor_tensor(
                        z[:], ch, 0.0, rs[:], ALU.bypass, ALU.mult)
                    e = work.tile([128, L], bf16, tag="e")
                    nc.scalar.activation(
                        e[:], z[:], ACT.Exp,
                        bias=gbt[:, H + h:H + h + 1], scale=gbt[:, h:h + 1])
                    # e' = e*mask; accum gives masked row-sum s (softmax denom)
                    nc.vector.scalar_tensor_tensor(
                        ch, e[:], 0.0, maskbt[:], ALU.bypass, ALU.mult,
                        accum_out=scol[:, h * NLC + lc:h * NLC + lc + 1])

                # phase 2 for this lc: transpose + AV per head
                for h in range(H):
                    pt_ps = psPT.tile([128, 8 * 128], bf16, tag="pt")
                    for mc in range(8):
                        nc.tensor.transpose(
                            pt_ps[:, mc * 128:(mc + 1) * 128],
                            ep[h][:, lc * L + mc * 128: lc * L + mc * 128 + 128],
                            idt[:])
                    pt_sb = work.tile([128, 8 * 128], bf16, tag="ptsb")
                    if h % 2 == 0:
                        nc.vector.tensor_copy(pt_sb[:], pt_ps[:])
                    else:
                        nc.scalar.copy(pt_sb[:], pt_ps[:])
                    av_ps = psAV.tile([65, 128], f32, tag="av")
                    for mc in range(8):
                        nc.tensor.matmul(
                            av_ps[:],
                            vxs[:, mc * (H * 65) + h * 65: mc * (H * 65) + (h + 1) * 65],
                            pt_sb[:, mc * 128:(mc + 1) * 128],
                            start=(mc == 0), stop=(mc == 7),
                        )
                    nc.vector.tensor_copy(o_sb[h][:, lc * 128:(lc + 1) * 128], av_ps[:])

            for h in range(H):
                nc.sync.dma_start(o_d[h], o_sb[h][:])

            # ---- w: wnum[m] = sum_h sum_l r[l] * e'[l, m] ----
            rcol = constp.tile([128, H * NLC], f32, tag="rcol")
            nc.vector.reciprocal(rcol[:], scol[:])
            rcolb = constp.tile([128, H * NLC], bf16, tag="rcolb")
            for lc in range(NLC):
                # zero out padded l rows: r *= valid_l (and cast to bf16)
                nc.vector.tensor_scalar_mul(
                    rcolb[:, lc::NLC], rcol[:, lc::NLC], vlt[:, lc:lc + 1])

            w_ps = psW.tile([1, L], f32, tag="wps")
            for h in range(H):
                for lc in range(NLC):
                    for mb in range(2):
                        nc.tensor.matmul(
                            w_ps[:, mb * 512:(mb + 1) * 512],
                            rcolb[:, h * NLC + lc: h * NLC + lc + 1],
                            ep[h][:, lc * L + mb * 512: lc * L + mb * 512 + 512],
                            start=(h == 0 and lc == 0), stop=(h == H - 1 and lc == NLC - 1),
                        )
            w_sb = constp.tile([1, L], f32, tag="wsb")
            nc.vector.tensor_copy(w_sb[:], w_ps[:])
            nc.sync.dma_start(wnum_d[:], w_sb[:])

    return nc


def _prep_core_inputs(K, Q, V, bx_packed, c):
    b, half = c // 2, c % 2
    l0 = half * LCORE
    kslice = np.ascontiguousarray(K[b, l0:l0 + LCORE, :].T)  # [D, LCORE]
    qTa = np.ascontiguousarray(Q[b].T)                        # [D, L]
    vxa = np.zeros((L, H * 65), dtype=np.float32)
    for h in range(H):
        vxa[:, h * 65:h * 65 + 64] = V[b][:, h * 64:(h + 1) * 64]
        vxa[:, h * 65 + 64] = 1.0
    valid_m = (~bx_packed[b]).astype(np.float32)              # [L]
    maskba = np.broadcast_to(valid_m[None, :], (128, L)).astype(BF)
    vla = np.empty((128, NLC), dtype=np.float32)
    for lc in range(NLC):
        vla[:, lc] = valid_m[l0 + lc * 128: l0 + (lc + 1) * 128]
    c32a = np.concatenate([qTa, kslice, (kslice * 0.125).astype(np.float32)], axis=1)
    return {
        "c32": np.ascontiguousarray(c32a),
        "vx": vxa.astype(BF),
        "maskb": np.ascontiguousarray(maskba),
        "vl": vla,
    }


def kernel(K, Q, V, doc_sizes, gamma, beta, pad_mask, bx_packed):
    global _NC
    K = np.asarray(K, dtype=np.float32)
    Q = np.asarray(Q, dtype=np.float32)
    V = np.asarray(V, dtype=np.float32)
    doc_sizes = np.asarray(doc_sizes, dtype=np.float32)
    gamma = np.asarray(gamma, dtype=np.float32)
    beta = np.asarray(beta, dtype=np.float32)
    bx_packed = np.asarray(bx_packed).astype(bool)

    gba = np.zeros((128, 2 * H + 1), dtype=np.float32)
    gba[:, 2 * H] = 1e-5
    gba[:, :H] = gamma[None, :]
    gba[:, H:2 * H] = beta[None, :]
    identa = np.eye(128, dtype=np.float32).astype(BF)

    in_maps = []
    for c in range(NCORES):
        m = _prep_core_inputs(K, Q, V, bx_packed, c)
        m["gb"] = gba
        m["identb"] = identa
        in_maps.append(m)

    if _NC is None:
        _NC = _build_nc()
    from concourse.bass_utils import run_bass_kernel_spmd
    res = run_bass_kernel_spmd(_NC, in_maps, list(range(NCORES))).results

    out = np.empty((B, L, D), dtype=np.float32)
    w = np.empty((B, L), dtype=np.float32)
    for b in range(B):
        for half in range(2):
            c = 2 * b + half
            o = np.asarray(res[c]["o"], dtype=np.float32)    # [H, 65, LCORE]
            s = o[:, 64, :]                                   # [H, LCORE]
            on = o[:, :64, :] / s[:, None, :]                 # [H, 64, LCORE]
            out[b, half * LCORE:(half + 1) * LCORE, :] = (
                on.transpose(2, 0, 1).reshape(LCORE, D))
        out[b, bx_packed[b], :] = 0.0
        wn = (np.asarray(res[2 * b]["wnum"])[0]
              + np.asarray(res[2 * b + 1]["wnum"])[0]).astype(np.float32)
        wp = wn / (H * doc_sizes[b, 0])
        wp = np.where(bx_packed[b], -np.inf, wp)
        wp = wp - wp.max()
        ew = np.exp(wp)
        w[b] = ew / ew.sum()
    return out, w[..., None].astype(np.float32)
